# revision 9
# baseline (speedup 1.0000x reference)
"""Multi-Latent Attention TRN2 kernel, v4: absorbed weights + hybrid sharding,
transposed U~ accumulation, projection/attention weaving, balanced engines.

Sharding: 2-way data parallel on batch x 4-way tensor parallel on heads.
Core c handles batch b = c // 4 and heads hg*4..hg*4+3 where hg = c % 4.
Each core computes a partial [S, D] output for its batch (contracting only
its heads' latent features); the host sums 4 partials per batch and adds
the folded output bias.

Weight absorption (exact algebra, done host-side in fp32):
  scores: s = (x Wq_h + bq_h) . (latk_h Wkr + bkr)
    per-row constants are softmax-invariant -> bkr, blk terms dropped;
    q~ = x (Wq_h Wkr^T) + bq_h Wkr^T; latk0 = x Wlk_h; contraction L=64.
  values: attn @ v_h = (attn latv0_h) @ Wvr + const row
    -> Wo_eff_h = Wvr @ Wo_h folded host-side; const row into bo_eff.

v4 structure (vs v3):
  * U~ is accumulated TRANSPOSED: psum [128 q, 65] per (pair, q-tile) with
    lhsT = exp'd score slab (stationary, FWL) and latv (+ones col) moving.
    Halves the U~ PE column count (output uses all 128 partitions) and the
    denominator lands at column 64 -> normalize is a per-partition
    tensor_scalar multiply; the old replicate-row matmul machinery is gone.
    A cheap PE transpose per (pair, q-tile) restores the [latent, token]
    layout the out-projection wants.
  * Chunk c+1's projections are woven into attention block c as PE filler,
    hiding the ScalarE exp latency; out-projections of blocks 0..2 fill the
    exp-bound final block, and block 3's out-projection chains per q-tile.
  * Startup DMA is KO-sliced so the first q~ matmul starts ~1us in.
  * Scores pairs run row-tiled on PE row groups (0,0)/(64,0) as in v3.
"""

import math
from contextlib import ExitStack

import numpy as np

import concourse.mybir as mybir
from concourse import bacc
from concourse.bass import ds, ts
from concourse.tile import TileContext

# Problem constants (hardcoded per contract).
B, S, D = 2, 2048, 2048
H, DK, DV, L = 16, 128, 128, 64
N_CORES = 8
HPC = 4                   # heads per core
NPAIR = 2                 # head pairs per core
SB = S                    # tokens per core (its batch)
KO = D // 128             # contraction k-tiles over D = 16
CHUNK = 512               # token chunk for input streaming
NCH = SB // CHUNK         # 4
QT = SB // 128            # 16 token tiles
LW = L + 1                # latv group width (64 latents + ones col)
NSLOT = 4                 # pt ring depth (k-tile slots per head)

F32 = mybir.dt.float32
BF16 = mybir.dt.bfloat16

INV_SQRT_DK = 1.0 / math.sqrt(DK)
EXPF = mybir.ActivationFunctionType.Exp


def build_kernel():
    nc = bacc.Bacc(trn_type="TRN2", debug=False, num_swdge_queues=2)

    # ---- DRAM I/O (all host-packed for contiguous DMA) ----
    xq = nc.dram_tensor("xq", [NCH, 128, KO, CHUNK], BF16, kind="ExternalInput")
    xk = nc.dram_tensor("xk", [NCH, 128, KO, CHUNK], BF16, kind="ExternalInput")
    xv = nc.dram_tensor("xv", [NCH, 128, KO, CHUNK], BF16, kind="ExternalInput")
    wq = nc.dram_tensor("wq", [NPAIR, 128, KO, 128], BF16, kind="ExternalInput")
    bq = nc.dram_tensor("bq", [128, NPAIR], F32, kind="ExternalInput")
    wlk = nc.dram_tensor("wlk", [128, KO, 128 * NPAIR], BF16, kind="ExternalInput")
    wlv = nc.dram_tensor("wlv", [128, KO, HPC * L], BF16, kind="ExternalInput")
    wo = nc.dram_tensor("wo", [128, NPAIR, D], BF16, kind="ExternalInput")
    outp = nc.dram_tensor("outp", [QT, 128, D], BF16, kind="ExternalOutput")

    with TileContext(nc) as tc, ExitStack() as ctx:
        ec = ctx.enter_context
        consts = ec(tc.tile_pool(name="consts", bufs=1))
        persist = ec(tc.tile_pool(name="persist", bufs=1))
        xpool = ec(tc.tile_pool(name="xpool", bufs=3))
        ptpool = ec(tc.tile_pool(name="ptpool", bufs=2))
        anpool = ec(tc.tile_pool(name="anpool", bufs=4))
        opool = ec(tc.tile_pool(name="opool", bufs=2))
        psa = ec(tc.tile_pool(name="psa", bufs=2, space="PSUM"))   # proj+outproj
        pss = ec(tc.tile_pool(name="pss", bufs=2, space="PSUM"))   # scores
        psu = ec(tc.tile_pool(name="psu", bufs=1, space="PSUM"))   # u0..u3 + tr

        # causal mask for a diagonal 128x128 block of P~^T: 1 where k <= q
        maskT = consts.tile([128, 128], BF16, tag="maskT")
        nc.gpsimd.memset(maskT, 1.0)
        nc.gpsimd.affine_select(
            out=maskT, in_=maskT, compare_op=mybir.AluOpType.is_ge,
            fill=0.0, base=0, pattern=[[1, 128]], channel_multiplier=-1,
        )


        # ---- persistent per-batch tensors ----
        qsb = persist.tile([128, NPAIR, SB], BF16, tag="qsb")
        ksb = persist.tile([128, NPAIR, SB], BF16, tag="ksb")
        vsb = persist.tile([128, QT, HPC * LW], BF16, tag="vsb")
        asb = persist.tile([128, NPAIR, SB], BF16, tag="asb")
        for h in range(HPC):
            nc.gpsimd.memset(vsb[:, :, LW * h + L : LW * h + L + 1], 1.0)

        # ---- weights + chunk-0 inputs, KO-sliced so the first q~ chain
        # ---- starts ~1us in, chasing the DMA stream ----
        wq_sb = consts.tile([128, NPAIR, KO, 128], BF16, tag="wq")
        bq_sb = consts.tile([128, NPAIR], F32, tag="bq")
        wlk_sb = consts.tile([128, KO, 128 * NPAIR], BF16, tag="wlk")
        wlv_sb = consts.tile([128, KO, HPC * L], BF16, tag="wlv")
        wo_sb = consts.tile([128, NPAIR, D], BF16, tag="wo")

        nc.gpsimd.dma_start(bq_sb, bq[:, :])
        xt = [[None] * 3 for _ in range(NCH)]  # [c][q/k/v]
        xt[0][0] = xpool.tile([128, KO, CHUNK], BF16, tag="x", name="xq_t")
        for ko in range(KO):
            nc.sync.dma_start(wq_sb[:, 0, ko, :], wq[0][:, ko, :])
            nc.scalar.dma_start(xt[0][0][:, ko, :], xq[0][:, ko, :])
        nc.sync.dma_start(wq_sb[:, 1], wq[1])
        xt[0][1] = xpool.tile([128, KO, CHUNK], BF16, tag="x", name="xk_t")
        for ko2 in range(0, KO, 4):
            nc.sync.dma_start(wlk_sb[:, ko2 : ko2 + 4, :],
                              wlk[:, ko2 : ko2 + 4, :])
            nc.scalar.dma_start(xt[0][1][:, ko2 : ko2 + 4, :],
                                xk[0][:, ko2 : ko2 + 4, :])
        xt[0][2] = xpool.tile([128, KO, CHUNK], BF16, tag="x", name="xv_t")
        nc.gpsimd.dma_start(wlv_sb, wlv[:, :, :])
        nc.gpsimd.dma_start(xt[0][2], xv[0])

        def load_chunk(c):
            xt[c][0] = xpool.tile([128, KO, CHUNK], BF16, tag="x", name="xq_t")
            nc.sync.dma_start(xt[c][0], xq[c])
            xt[c][1] = xpool.tile([128, KO, CHUNK], BF16, tag="x", name="xk_t")
            nc.scalar.dma_start(xt[c][1], xk[c])
            xt[c][2] = xpool.tile([128, KO, CHUNK], BF16, tag="x", name="xv_t")
            nc.gpsimd.dma_start(xt[c][2], xv[c])

        load_chunk(1)  # lands during chunk-0 projections / block 0
        nc.gpsimd.dma_start(wo_sb, wo[:, :, :])

        # ---- unit builders (called inline or dispensed as PE filler) ----
        def unit_q(c, m):
            csl = ds(c * CHUNK, CHUNK)
            ps = psa.tile([128, 512], F32, tag="s", name="ps_q")
            for ko in range(KO):
                nc.tensor.matmul(
                    ps, wq_sb[:, m, ko, :], xt[c][0][:, ko, :],
                    start=(ko == 0), stop=(ko == KO - 1),
                )
            nc.vector.tensor_scalar_add(
                qsb[:, m, csl], ps, bq_sb[:, m : m + 1])

        def unit_k(c, m):
            csl = ds(c * CHUNK, CHUNK)
            ps = psa.tile([128, 512], F32, tag="s", name="ps_k")
            for ko in range(KO):
                nc.tensor.matmul(
                    ps, wlk_sb[:, ko, ts(m, 128)], xt[c][1][:, ko, :],
                    start=(ko == 0), stop=(ko == KO - 1),
                )
            nc.vector.tensor_copy(out=ksb[:, m, csl], in_=ps)

        def unit_v(c, tl):
            tt = c * 4 + tl
            ps = psa.tile([128, 512], F32, tag="s", name="ps_v")
            for ko in range(KO):
                nc.tensor.matmul(
                    ps[:, : HPC * L], xt[c][2][:, ko, ts(tl, 128)],
                    wlv_sb[:, ko, :],
                    start=(ko == 0), stop=(ko == KO - 1),
                )
            for h in range(HPC):
                nc.vector.tensor_copy(
                    out=vsb[:, tt, ds(LW * h, L)],
                    in_=ps[:, ds(L * h, L)],
                )

        def proj_units(c):
            return [
                lambda: unit_q(c, 0), lambda: unit_q(c, 1),
                lambda: unit_k(c, 0), lambda: unit_k(c, 1),
                lambda: unit_v(c, 0), lambda: unit_v(c, 1),
                lambda: unit_v(c, 2), lambda: unit_v(c, 3),
            ]

        def out_proj_unit(Q, tl):
            tt = Q * 4 + tl
            o_sb = opool.tile([128, D], BF16, tag="o", name="o_sb")
            for dc2 in range(2):
                ps_f = [psa.tile([128, 512], F32, tag="s", name="ps_f")
                        for _ in range(2)]
                for kk in range(NPAIR):
                    for i in range(2):  # one LDWEIGHTS feeds two matmuls
                        nc.tensor.matmul(
                            ps_f[i], asb[:, kk, ts(tt, 128)],
                            wo_sb[:, kk, ts(2 * dc2 + i, 512)],
                            start=(kk == 0), stop=(kk == NPAIR - 1),
                        )
                for i in range(2):
                    nc.vector.tensor_copy(out=o_sb[:, ts(2 * dc2 + i, 512)],
                                          in_=ps_f[i])
            nc.sync.dma_start(outp[tt], o_sb)

        # ---- attention block with filler weaving ----
        def att_block(Q, filler, inline_oproj=False):
            qsl = ds(Q * 512, 512)
            jmax = 4 * Q + 4
            nun = 2 * jmax
            state = {"f": 0, "j": 0}

            def dispense():
                want = (state["j"] * len(filler)) // nun
                while state["f"] < want:
                    filler[state["f"]]()
                    state["f"] += 1

            for p in range(NPAIR):
                pt = [ptpool.tile([128, NSLOT, 512], BF16, tag=f"pt{r}",
                                  name=f"pt{r}") for r in range(2)]
                us = [psu.tile([128, 130], F32, tag=f"u{qt}", name="us")
                      for qt in range(4)]

                def u_chase(j):
                    # start=True resets the WHOLE psum bank, so only the
                    # very first matmul per us[qt] bank may carry it; the
                    # r=1 range's first write lands via clear has_written
                    # bits (accumulate-with-clear-bit == plain write).
                    slot = j % NSLOT
                    for r in range(2):
                        for qt in range(4):
                            nc.tensor.matmul(
                                us[qt][:, ds(65 * r, 65)],
                                pt[r][:, slot, ts(qt, 128)],
                                vsb[:, j, ds(LW * (2 * p + r), LW)],
                                start=(j == 0 and r == 0),
                                stop=(j == jmax - 1 and r == 1),
                                skip_group_check=True,
                            )

                for j in range(jmax):
                    slot = j % NSLOT
                    sg = [pss.tile([128, 512], F32, tag="sg", name="sg")
                          for _ in range(2)]
                    for r in range(2):
                        rs = slice(64 * r, 64 * r + 64)
                        nc.tensor.matmul(
                            sg[r], ksb[rs, p, ts(j, 128)], qsb[rs, p, qsl],
                            start=True, stop=True,
                        )
                    for r in range(2):
                        nc.scalar.activation(
                            pt[r][:, slot, :], sg[r], EXPF,
                            scale=INV_SQRT_DK,
                        )
                    i = j - 4 * Q
                    if i >= 0:  # diagonal k-tile: causal masking (GpSimd)
                        for r in range(2):
                            if i > 0:
                                nc.gpsimd.memset(
                                    pt[r][:, slot, ds(0, 128 * i)], 0.0)
                            nc.gpsimd.tensor_tensor(
                                pt[r][:, slot, ds(128 * i, 128)],
                                pt[r][:, slot, ds(128 * i, 128)],
                                maskT, mybir.AluOpType.mult,
                            )
                    if j > 0:
                        u_chase(j - 1)
                    state["j"] += 1
                    dispense()
                u_chase(jmax - 1)

                # normalize (den at cols 64 / 129), then DMA-XBAR transpose
                # [q, lat] -> asb [lat, q] (no PE, no psum)
                rcp = anpool.tile([128, 8], F32, tag="rcp", name="rcp")
                for qt in range(4):
                    nc.vector.reciprocal(
                        rcp[:, 2 * qt : 2 * qt + 1], us[qt][:, 64:65])
                    nc.vector.reciprocal(
                        rcp[:, 2 * qt + 1 : 2 * qt + 2], us[qt][:, 129:130])
                for qt in range(4):
                    an = anpool.tile([128, 128], BF16, tag="an", name="an")
                    nc.vector.tensor_scalar_mul(
                        an[:, 0:64], us[qt][:, 0:64],
                        rcp[:, 2 * qt : 2 * qt + 1])
                    nc.vector.tensor_scalar_mul(
                        an[:, 64:128], us[qt][:, 65:129],
                        rcp[:, 2 * qt + 1 : 2 * qt + 2])
                    nc.scalar.dma_start(
                        asb[:, p, ds(Q * 512 + qt * 128, 128)], an,
                        transpose=True)
                if inline_oproj and p == NPAIR - 1:
                    for qt in range(4):
                        out_proj_unit(Q, qt)

            while state["f"] < len(filler):
                filler[state["f"]]()
                state["f"] += 1

        # ---- schedule ----
        # chunk-0 q~/latk inline (DMA-paced); latv(0) weaves into block 0
        unit_q(0, 0)
        unit_q(0, 1)
        unit_k(0, 0)
        unit_k(0, 1)

        # block 0: weave chunk-0 latv + chunk-1 projections; start x2 DMA
        load_chunk(2)
        att_block(0, [lambda tl=tl: unit_v(0, tl) for tl in range(4)]
                  + proj_units(1))
        # block 1: weave chunk-2 projections; start x3 DMA
        load_chunk(3)
        att_block(1, proj_units(2))
        # block 2: weave chunk-3 projections
        att_block(2, proj_units(3))
        # block 3: exp-bound; fill with all deferred out-projections,
        # chain block-3's own out-projection per q-tile
        fill3 = [(lambda Q=Q, tl=tl: out_proj_unit(Q, tl))
                 for Q in range(3) for tl in range(4)]
        att_block(3, fill3, inline_oproj=True)

    nc.finalize()
    return nc


_NC_CACHE = None


def _get_nc():
    global _NC_CACHE
    if _NC_CACHE is None:
        _NC_CACHE = build_kernel()
    return _NC_CACHE


def _pack_xT(Xb, bf16):
    # Xb [S, D] fp32 -> X^T packed [NCH, 128, KO, CHUNK] (d = ko*128 + p)
    xt = np.asarray(Xb).T.reshape(KO, 128, NCH, CHUNK)
    return np.ascontiguousarray(xt.transpose(2, 1, 0, 3).astype(bf16))


def _prep_in_maps(queries, keys, values, Wq, bq, Wlk, blk, Wlv, blv,
                  Wkr, bkr, Wvr, bvr, Wo, bo):
    import ml_dtypes

    bf16 = ml_dtypes.bfloat16
    f = np.float32
    Wq, bq, Wlk, Wlv = (np.asarray(a, f) for a in (Wq, bq, Wlk, Wlv))
    Wkr, Wvr, Wo = (np.asarray(a, f) for a in (Wkr, Wvr, Wo))

    # host-side absorption folds (exact algebra)
    # Wq_eff_h = Wq_h @ Wkr^T [D, L]; bq_eff_h = bq_h @ Wkr^T
    WqH = Wq.reshape(D, H, DK)
    Wq_eff = np.einsum("dhk,lk->dhl", WqH, Wkr).reshape(D, H * L)
    bq_eff = (bq.reshape(H, DK) @ Wkr.T).reshape(H * L)
    # Wo_eff_h = Wvr @ Wo_h [L, D]
    WoH = Wo.reshape(H, DV, D)
    Wo_eff = np.einsum("lk,hkd->hld", Wvr, WoH).reshape(H * L, D)

    in_maps = []
    for c in range(N_CORES):
        b, hg = c // 4, c % 4
        hsl = slice(hg * 4 * L, (hg + 1) * 4 * L)     # 4 heads' latent cols

        xq_c = _pack_xT(queries[b], bf16)
        xk_c = _pack_xT(keys[b], bf16)
        xv_c = _pack_xT(values[b], bf16)

        wq_c = np.ascontiguousarray(
            Wq_eff[:, hsl].reshape(KO, 128, NPAIR, 128)
            .transpose(2, 1, 0, 3).astype(bf16))
        bq_c = np.ascontiguousarray(
            bq_eff[hsl].reshape(NPAIR, 128).T, f)
        wlk_c = np.ascontiguousarray(
            Wlk[:, hsl].reshape(KO, 128, 256).transpose(1, 0, 2).astype(bf16))
        wlv_c = np.ascontiguousarray(
            Wlv[:, hsl].reshape(KO, 128, 256).transpose(1, 0, 2).astype(bf16))
        wo_c = np.ascontiguousarray(
            Wo_eff[hsl, :].reshape(NPAIR, 128, D).transpose(1, 0, 2)
            .astype(bf16))

        in_maps.append({
            "xq": xq_c, "xk": xk_c, "xv": xv_c,
            "wq": wq_c, "bq": bq_c, "wlk": wlk_c, "wlv": wlv_c, "wo": wo_c,
        })
    return in_maps


def _assemble(results, inputs):
    f64 = np.float64
    blv = np.asarray(inputs["blv"], f64).reshape(H, L)
    bvr = np.asarray(inputs["bvr"], f64)
    Wvr = np.asarray(inputs["Wvr"], f64)
    WoH = np.asarray(inputs["Wo"], f64).reshape(H, DV, D)
    bo_eff = np.asarray(inputs["bo"], f64).copy()
    for h in range(H):
        bo_eff += (blv[h] @ Wvr + bvr) @ WoH[h]

    out = np.zeros((B, S, D), f64)
    for c, rmap in enumerate(results):
        out[c // 4] += rmap["outp"].reshape(S, D).astype(f64)
    out += bo_eff
    return out.astype(np.float32)


def kernel(**inputs):
    from concourse.bass_utils import run_bass_kernel_spmd

    nc = _get_nc()
    in_maps = _prep_in_maps(**inputs)
    res = run_bass_kernel_spmd(
        nc, in_maps, core_ids=list(range(N_CORES)), trace=False
    )
    return _assemble(res.results, inputs)


if __name__ == "__main__":
    nc = build_kernel()
    print("built ok, instructions:", len(nc.inst_map))


# revision 12
# speedup vs baseline: 1.6068x; 1.6068x over previous
"""Multi-Latent Attention TRN2 kernel, v4: absorbed weights + hybrid sharding,
transposed U~ accumulation, projection/attention weaving, balanced engines.

Sharding: 2-way data parallel on batch x 4-way tensor parallel on heads.
Core c handles batch b = c // 4 and heads hg*4..hg*4+3 where hg = c % 4.
Each core computes a partial [S, D] output for its batch (contracting only
its heads' latent features); the host sums 4 partials per batch and adds
the folded output bias.

Weight absorption (exact algebra, done host-side in fp32):
  scores: s = (x Wq_h + bq_h) . (latk_h Wkr + bkr)
    per-row constants are softmax-invariant -> bkr, blk terms dropped;
    q~ = x (Wq_h Wkr^T) + bq_h Wkr^T; latk0 = x Wlk_h; contraction L=64.
  values: attn @ v_h = (attn latv0_h) @ Wvr + const row
    -> Wo_eff_h = Wvr @ Wo_h folded host-side; const row into bo_eff.

v4 structure (vs v3):
  * U~ is accumulated TRANSPOSED: psum [128 q, 65] per (pair, q-tile) with
    lhsT = exp'd score slab (stationary, FWL) and latv (+ones col) moving.
    Halves the U~ PE column count (output uses all 128 partitions) and the
    denominator lands at column 64 -> normalize is a per-partition
    tensor_scalar multiply; the old replicate-row matmul machinery is gone.
    A cheap PE transpose per (pair, q-tile) restores the [latent, token]
    layout the out-projection wants.
  * Chunk c+1's projections are woven into attention block c as PE filler,
    hiding the ScalarE exp latency; out-projections of blocks 0..2 fill the
    exp-bound final block, and block 3's out-projection chains per q-tile.
  * Startup DMA is KO-sliced so the first q~ matmul starts ~1us in.
  * Scores pairs run row-tiled on PE row groups (0,0)/(64,0) as in v3.
"""

import math
from contextlib import ExitStack

import numpy as np

import concourse.mybir as mybir
from concourse import bacc
from concourse.bass import ds, ts
from concourse.masks import make_identity
from concourse.tile import TileContext

# Problem constants (hardcoded per contract).
B, S, D = 2, 2048, 2048
H, DK, DV, L = 16, 128, 128, 64
N_CORES = 8
HPC = 4                   # heads per core
NPAIR = 2                 # head pairs per core
SB = S                    # tokens per core (its batch)
KO = D // 128             # contraction k-tiles over D = 16
CHUNK = 512               # token chunk for input streaming
NCH = SB // CHUNK         # 4
QT = SB // 128            # 16 token tiles
LW = L + 1                # latv group width (64 latents + ones col)
NSLOT = 4                 # pt ring depth (k-tile slots per head)

F32 = mybir.dt.float32
BF16 = mybir.dt.bfloat16

INV_SQRT_DK = 1.0 / math.sqrt(DK)
EXPF = mybir.ActivationFunctionType.Exp


def build_kernel():
    nc = bacc.Bacc(trn_type="TRN2", debug=False, num_swdge_queues=2)

    # ---- DRAM I/O (all host-packed for contiguous DMA) ----
    xq = nc.dram_tensor("xq", [NCH, 128, KO, CHUNK], BF16, kind="ExternalInput")
    xk = nc.dram_tensor("xk", [NCH, 128, KO, CHUNK], BF16, kind="ExternalInput")
    xv = nc.dram_tensor("xv", [NCH, 128, KO, CHUNK], BF16, kind="ExternalInput")
    wq = nc.dram_tensor("wq", [NPAIR, 128, KO, 128], BF16, kind="ExternalInput")
    bq = nc.dram_tensor("bq", [128, NPAIR], F32, kind="ExternalInput")
    wlk = nc.dram_tensor("wlk", [128, KO, 128 * NPAIR], BF16, kind="ExternalInput")
    wlv = nc.dram_tensor("wlv", [128, KO, HPC * L], BF16, kind="ExternalInput")
    wo = nc.dram_tensor("wo", [128, NPAIR, D], BF16, kind="ExternalInput")
    outp = nc.dram_tensor("outp", [QT, 128, D], BF16, kind="ExternalOutput")

    with TileContext(nc) as tc, ExitStack() as ctx:
        ec = ctx.enter_context
        consts = ec(tc.tile_pool(name="consts", bufs=1))
        persist = ec(tc.tile_pool(name="persist", bufs=1))
        xpool = ec(tc.tile_pool(name="xpool", bufs=3))
        ptpool = ec(tc.tile_pool(name="ptpool", bufs=2))
        anpool = ec(tc.tile_pool(name="anpool", bufs=4))
        opool = ec(tc.tile_pool(name="opool", bufs=2))
        psa = ec(tc.tile_pool(name="psa", bufs=2, space="PSUM"))   # proj+outproj
        pss = ec(tc.tile_pool(name="pss", bufs=2, space="PSUM"))   # scores
        psu = ec(tc.tile_pool(name="psu", bufs=1, space="PSUM"))   # u0..u3 + tr

        # causal mask for a diagonal 128x128 block of P~^T: 1 where k <= q
        maskT = consts.tile([128, 128], BF16, tag="maskT")
        nc.gpsimd.memset(maskT, 1.0)
        nc.gpsimd.affine_select(
            out=maskT, in_=maskT, compare_op=mybir.AluOpType.is_ge,
            fill=0.0, base=0, pattern=[[1, 128]], channel_multiplier=-1,
        )
        ident = consts.tile([128, 128], BF16, tag="ident")
        make_identity(nc, ident)


        # ---- persistent per-batch tensors ----
        qsb = persist.tile([128, NPAIR, SB], BF16, tag="qsb")
        ksb = persist.tile([128, NPAIR, SB], BF16, tag="ksb")
        vsb = persist.tile([128, QT, HPC * LW], BF16, tag="vsb")
        asb = persist.tile([128, NPAIR, SB], BF16, tag="asb")
        for h in range(HPC):
            nc.gpsimd.memset(vsb[:, :, LW * h + L : LW * h + L + 1], 1.0)

        # ---- weights + chunk-0 inputs, KO-sliced so the first q~ chain
        # ---- starts ~1us in, chasing the DMA stream ----
        wq_sb = consts.tile([128, NPAIR, KO, 128], BF16, tag="wq")
        bq_sb = consts.tile([128, NPAIR], F32, tag="bq")
        wlk_sb = consts.tile([128, KO, 128 * NPAIR], BF16, tag="wlk")
        wlv_sb = consts.tile([128, KO, HPC * L], BF16, tag="wlv")
        wo_sb = consts.tile([128, NPAIR, D], BF16, tag="wo")

        nc.gpsimd.dma_start(bq_sb, bq[:, :])
        xt = [[None] * 3 for _ in range(NCH)]  # [c][q/k/v]
        xt[0][0] = xpool.tile([128, KO, CHUNK], BF16, tag="x", name="xq_t")
        for ko in range(KO):
            nc.sync.dma_start(wq_sb[:, 0, ko, :], wq[0][:, ko, :])
            nc.scalar.dma_start(xt[0][0][:, ko, :], xq[0][:, ko, :])
        nc.sync.dma_start(wq_sb[:, 1], wq[1])
        xt[0][1] = xpool.tile([128, KO, CHUNK], BF16, tag="x", name="xk_t")
        for ko2 in range(0, KO, 4):
            nc.sync.dma_start(wlk_sb[:, ko2 : ko2 + 4, :],
                              wlk[:, ko2 : ko2 + 4, :])
            nc.scalar.dma_start(xt[0][1][:, ko2 : ko2 + 4, :],
                                xk[0][:, ko2 : ko2 + 4, :])
        xt[0][2] = xpool.tile([128, KO, CHUNK], BF16, tag="x", name="xv_t")
        nc.gpsimd.dma_start(wlv_sb, wlv[:, :, :])
        nc.gpsimd.dma_start(xt[0][2], xv[0])

        def load_chunk(c):
            xt[c][0] = xpool.tile([128, KO, CHUNK], BF16, tag="x", name="xq_t")
            nc.sync.dma_start(xt[c][0], xq[c])
            xt[c][1] = xpool.tile([128, KO, CHUNK], BF16, tag="x", name="xk_t")
            nc.scalar.dma_start(xt[c][1], xk[c])
            xt[c][2] = xpool.tile([128, KO, CHUNK], BF16, tag="x", name="xv_t")
            nc.gpsimd.dma_start(xt[c][2], xv[c])

        load_chunk(1)  # lands during chunk-0 projections / block 0
        nc.gpsimd.dma_start(wo_sb, wo[:, :, :])

        # ---- unit builders (called inline or dispensed as PE filler) ----
        def unit_q(c, m):
            csl = ds(c * CHUNK, CHUNK)
            ps = psa.tile([128, 512], F32, tag="s", name="ps_q")
            for ko in range(KO):
                nc.tensor.matmul(
                    ps, wq_sb[:, m, ko, :], xt[c][0][:, ko, :],
                    start=(ko == 0), stop=(ko == KO - 1),
                )
            nc.vector.tensor_scalar_add(
                qsb[:, m, csl], ps, bq_sb[:, m : m + 1])

        def unit_k(c, m):
            csl = ds(c * CHUNK, CHUNK)
            ps = psa.tile([128, 512], F32, tag="s", name="ps_k")
            for ko in range(KO):
                nc.tensor.matmul(
                    ps, wlk_sb[:, ko, ts(m, 128)], xt[c][1][:, ko, :],
                    start=(ko == 0), stop=(ko == KO - 1),
                )
            nc.vector.tensor_copy(out=ksb[:, m, csl], in_=ps)

        def unit_v(c, tl):
            tt = c * 4 + tl
            ps = psa.tile([128, 512], F32, tag="s", name="ps_v")
            for ko in range(KO):
                nc.tensor.matmul(
                    ps[:, : HPC * L], xt[c][2][:, ko, ts(tl, 128)],
                    wlv_sb[:, ko, :],
                    start=(ko == 0), stop=(ko == KO - 1),
                )
            for h in range(HPC):
                nc.vector.tensor_copy(
                    out=vsb[:, tt, ds(LW * h, L)],
                    in_=ps[:, ds(L * h, L)],
                )

        def proj_units(c):
            return [
                lambda: unit_q(c, 0), lambda: unit_q(c, 1),
                lambda: unit_k(c, 0), lambda: unit_k(c, 1),
                lambda: unit_v(c, 0), lambda: unit_v(c, 1),
                lambda: unit_v(c, 2), lambda: unit_v(c, 3),
            ]

        def out_proj_unit(Q, tl):
            tt = Q * 4 + tl
            o_sb = opool.tile([128, D], BF16, tag="o", name="o_sb")
            for dc2 in range(2):
                ps_f = [psa.tile([128, 512], F32, tag="s", name="ps_f")
                        for _ in range(2)]
                for kk in range(NPAIR):
                    for i in range(2):  # one LDWEIGHTS feeds two matmuls
                        nc.tensor.matmul(
                            ps_f[i], asb[:, kk, ts(tt, 128)],
                            wo_sb[:, kk, ts(2 * dc2 + i, 512)],
                            start=(kk == 0), stop=(kk == NPAIR - 1),
                        )
                for i in range(2):
                    nc.vector.tensor_copy(out=o_sb[:, ts(2 * dc2 + i, 512)],
                                          in_=ps_f[i])
            nc.sync.dma_start(outp[tt], o_sb)

        # ---- attention block with filler weaving ----
        def att_block(Q, filler, inline_oproj=False):
            qsl = ds(Q * 512, 512)
            jmax = 4 * Q + 4
            nun = 2 * jmax
            state = {"f": 0, "j": 0}

            def dispense():
                want = (state["j"] * len(filler)) // nun
                while state["f"] < want:
                    filler[state["f"]]()
                    state["f"] += 1

            for p in range(NPAIR):
                pt = [ptpool.tile([128, NSLOT, 512], BF16, tag=f"pt{r}",
                                  name=f"pt{r}") for r in range(2)]
                us = [psu.tile([128, 130], F32, tag=f"u{qt}", name="us")
                      for qt in range(4)]

                def u_chase(j):
                    # start=True resets the WHOLE psum bank, so only the
                    # very first matmul per us[qt] bank may carry it; the
                    # r=1 range's first write lands via clear has_written
                    # bits (accumulate-with-clear-bit == plain write).
                    slot = j % NSLOT
                    for r in range(2):
                        for qt in range(4):
                            nc.tensor.matmul(
                                us[qt][:, ds(65 * r, 65)],
                                pt[r][:, slot, ts(qt, 128)],
                                vsb[:, j, ds(LW * (2 * p + r), LW)],
                                start=(j == 0 and r == 0),
                                stop=(j == jmax - 1 and r == 1),
                                skip_group_check=True,
                            )

                for j in range(jmax):
                    slot = j % NSLOT
                    sg = [pss.tile([128, 512], F32, tag="sg", name="sg")
                          for _ in range(2)]
                    for r in range(2):
                        rs = slice(64 * r, 64 * r + 64)
                        nc.tensor.matmul(
                            sg[r], ksb[rs, p, ts(j, 128)], qsb[rs, p, qsl],
                            start=True, stop=True,
                        )
                    for r in range(2):
                        nc.scalar.activation(
                            pt[r][:, slot, :], sg[r], EXPF,
                            scale=INV_SQRT_DK,
                        )
                    i = j - 4 * Q
                    if i >= 0:  # diagonal k-tile: causal masking (GpSimd)
                        for r in range(2):
                            if i > 0:
                                nc.gpsimd.memset(
                                    pt[r][:, slot, ds(0, 128 * i)], 0.0)
                            nc.gpsimd.tensor_tensor(
                                pt[r][:, slot, ds(128 * i, 128)],
                                pt[r][:, slot, ds(128 * i, 128)],
                                maskT, mybir.AluOpType.mult,
                            )
                    if j > 0:
                        u_chase(j - 1)
                    state["j"] += 1
                    dispense()
                u_chase(jmax - 1)

                # normalize (den at cols 64 / 129): all norms first so the
                # PE transposes then run back-to-back without DVE stalls
                rcp = anpool.tile([128, 8], F32, tag="rcp", name="rcp")
                for qt in range(4):
                    nc.vector.reciprocal(
                        rcp[:, 2 * qt : 2 * qt + 1], us[qt][:, 64:65])
                    nc.vector.reciprocal(
                        rcp[:, 2 * qt + 1 : 2 * qt + 2], us[qt][:, 129:130])
                ans = []
                for qt in range(4):
                    an = anpool.tile([128, 128], BF16, tag="an", name="an")
                    nc.vector.tensor_scalar_mul(
                        an[:, 0:64], us[qt][:, 0:64],
                        rcp[:, 2 * qt : 2 * qt + 1])
                    nc.vector.tensor_scalar_mul(
                        an[:, 64:128], us[qt][:, 65:129],
                        rcp[:, 2 * qt + 1 : 2 * qt + 2])
                    ans.append(an)
                trs = []
                for qt in range(4):
                    tr = psu.tile([128, 128], BF16, tag=f"u{qt}", name="tr")
                    nc.tensor.transpose(tr, ans[qt], ident)
                    trs.append(tr)
                for qt in range(4):
                    nc.vector.tensor_copy(
                        out=asb[:, p, ds(Q * 512 + qt * 128, 128)],
                        in_=trs[qt])
                if inline_oproj and p == NPAIR - 1:
                    for qt in range(4):
                        out_proj_unit(Q, qt)

            while state["f"] < len(filler):
                filler[state["f"]]()
                state["f"] += 1

        # ---- schedule ----
        # chunk-0 q~/latk inline (DMA-paced); latv(0) weaves into block 0
        unit_q(0, 0)
        unit_q(0, 1)
        unit_k(0, 0)
        unit_k(0, 1)

        # block 0: weave chunk-0 latv + chunk-1 projections; start x2 DMA
        load_chunk(2)
        att_block(0, [lambda tl=tl: unit_v(0, tl) for tl in range(4)]
                  + proj_units(1))
        # block 1: weave chunk-2 projections; start x3 DMA
        load_chunk(3)
        att_block(1, proj_units(2))
        # block 2: weave chunk-3 projections
        att_block(2, proj_units(3))
        # block 3: exp-bound; fill with all deferred out-projections,
        # chain block-3's own out-projection per q-tile
        fill3 = [(lambda Q=Q, tl=tl: out_proj_unit(Q, tl))
                 for Q in range(3) for tl in range(4)]
        att_block(3, fill3, inline_oproj=True)

    nc.finalize()
    return nc


_NC_CACHE = None


def _get_nc():
    global _NC_CACHE
    if _NC_CACHE is None:
        _NC_CACHE = build_kernel()
    return _NC_CACHE


def _pack_xT(Xb, bf16):
    # Xb [S, D] fp32 -> X^T packed [NCH, 128, KO, CHUNK] (d = ko*128 + p)
    xt = np.asarray(Xb).T.reshape(KO, 128, NCH, CHUNK)
    return np.ascontiguousarray(xt.transpose(2, 1, 0, 3).astype(bf16))


def _prep_in_maps(queries, keys, values, Wq, bq, Wlk, blk, Wlv, blv,
                  Wkr, bkr, Wvr, bvr, Wo, bo):
    import ml_dtypes

    bf16 = ml_dtypes.bfloat16
    f = np.float32
    Wq, bq, Wlk, Wlv = (np.asarray(a, f) for a in (Wq, bq, Wlk, Wlv))
    Wkr, Wvr, Wo = (np.asarray(a, f) for a in (Wkr, Wvr, Wo))

    # host-side absorption folds (exact algebra)
    # Wq_eff_h = Wq_h @ Wkr^T [D, L]; bq_eff_h = bq_h @ Wkr^T
    WqH = Wq.reshape(D, H, DK)
    Wq_eff = np.einsum("dhk,lk->dhl", WqH, Wkr).reshape(D, H * L)
    bq_eff = (bq.reshape(H, DK) @ Wkr.T).reshape(H * L)
    # Wo_eff_h = Wvr @ Wo_h [L, D]
    WoH = Wo.reshape(H, DV, D)
    Wo_eff = np.einsum("lk,hkd->hld", Wvr, WoH).reshape(H * L, D)

    in_maps = []
    for c in range(N_CORES):
        b, hg = c // 4, c % 4
        hsl = slice(hg * 4 * L, (hg + 1) * 4 * L)     # 4 heads' latent cols

        xq_c = _pack_xT(queries[b], bf16)
        xk_c = _pack_xT(keys[b], bf16)
        xv_c = _pack_xT(values[b], bf16)

        wq_c = np.ascontiguousarray(
            Wq_eff[:, hsl].reshape(KO, 128, NPAIR, 128)
            .transpose(2, 1, 0, 3).astype(bf16))
        bq_c = np.ascontiguousarray(
            bq_eff[hsl].reshape(NPAIR, 128).T, f)
        wlk_c = np.ascontiguousarray(
            Wlk[:, hsl].reshape(KO, 128, 256).transpose(1, 0, 2).astype(bf16))
        wlv_c = np.ascontiguousarray(
            Wlv[:, hsl].reshape(KO, 128, 256).transpose(1, 0, 2).astype(bf16))
        wo_c = np.ascontiguousarray(
            Wo_eff[hsl, :].reshape(NPAIR, 128, D).transpose(1, 0, 2)
            .astype(bf16))

        in_maps.append({
            "xq": xq_c, "xk": xk_c, "xv": xv_c,
            "wq": wq_c, "bq": bq_c, "wlk": wlk_c, "wlv": wlv_c, "wo": wo_c,
        })
    return in_maps


def _assemble(results, inputs):
    f64 = np.float64
    blv = np.asarray(inputs["blv"], f64).reshape(H, L)
    bvr = np.asarray(inputs["bvr"], f64)
    Wvr = np.asarray(inputs["Wvr"], f64)
    WoH = np.asarray(inputs["Wo"], f64).reshape(H, DV, D)
    bo_eff = np.asarray(inputs["bo"], f64).copy()
    for h in range(H):
        bo_eff += (blv[h] @ Wvr + bvr) @ WoH[h]

    out = np.zeros((B, S, D), f64)
    for c, rmap in enumerate(results):
        out[c // 4] += rmap["outp"].reshape(S, D).astype(f64)
    out += bo_eff
    return out.astype(np.float32)


def kernel(**inputs):
    from concourse.bass_utils import run_bass_kernel_spmd

    nc = _get_nc()
    in_maps = _prep_in_maps(**inputs)
    res = run_bass_kernel_spmd(
        nc, in_maps, core_ids=list(range(N_CORES)), trace=False
    )
    return _assemble(res.results, inputs)


if __name__ == "__main__":
    nc = build_kernel()
    print("built ok, instructions:", len(nc.inst_map))
